# revision 13
# baseline (speedup 1.0000x reference)
"""Trainium2 Bass kernel for DecoderLinear_for_EffectiveLP_multiclass.

Math (reference):
    src = x @ w_src.T + b_src            # [N]
    dst = x @ w_dst.T + b_dst            # [N]
    s_ij = sigmoid(src[i] + dst[j])      # [N, N]
    channels: p_nb=(1-s_ij)(1-s_ji), p_pu=s_ij(1-s_ji),
              p_pb=s_ij*s_ji,        p_nu=(1-s_ij)s_ji
    out = log(clip(probs, 1e-10, 1))     # [N*N, 4]

On-device identities (clip never fires: |z| <= ~5 so min prob >> 1e-10):
    sp(z) = softplus(z);  z1 = src_i + dst_j;  z2 = dst_i + src_j
    -ch0 = sp1+sp2; -ch1 = sp1+sp2-z1; -ch3 = sp1+sp2-z2; -ch2 = sp1+sp2-z1-z2
The device writes the NEGATED channels in bf16; the host flips the sign
during the bf16->f32 conversion. The rel-err budget (2e-2) dwarfs bf16
rounding (~0.5% on the Frobenius norm).

Design (per core: 512 rows x 4096 j, 8 cores row-blockwise):
  - Host passes xT = x.T (bf16) so the projections run on the TENSOR engine:
    lhsT = w column [128d, 1], rhs = xT slice [128d, n<=512] -> PSUM row
    [1, n] accumulated over the two 128-d halves. No DVE reductions at all.
  - Row->plane broadcast is a single ones-vector matmul per 512-col chunk:
    out[p, j] = ones[1,p]^T @ row[1, j] (k=1). No transposes, no selectors.
  - softplus on ACT in ONE op per tile: sp1 = Ln(esc_rb * ed + 1) where
    ed = exp(dst-plane) (per-jc ACT Exp from PSUM) and esc_rb = exp(bias col)
    enters via the per-partition `scale` operand of ACTIVATE.
  - Per-core bias columns come from xbT (own 512 rows of xT): PE projects
    them to rows, then 8 tiny [1,128]->[128,1] transpose-matmuls make the
    per-partition columns. +(b_src+b_dst) is folded into the columns.
  - DVE per tile: 2x tensor_scalar (z1, z2; bf16 4x mode) + 4x tensor_tensor
    (bf16 2x mode) writing the four NEGATED channel planes contiguously.
  - Output tile [128, 4, 2048] bf16 (2 MiB) -> HBM; 16 tiles; jc-outer order
    so the j=0 planes gate only the first 4 tiles.

Measured op costs backing this schedule (this container, [128,2048] f32/bf16):
  DVE TT bf16 1211ns (2x), TS bf16 ~594ns (4x), STT always 1x (~2.3us);
  ACT 2.0-2.1us any func; PE matmul ~420ns + 320ns ldweights; per-core DMA
  ~335GB/s. Per-jc budget at 335GB/s is ~28us; DVE ~26.4, ACT ~26.3, PE ~18.
"""

import numpy as np
import ml_dtypes

import concourse.bass as bass
import concourse.mybir as mybir
from concourse.bass_utils import run_bass_kernel_spmd

N = 4096
D = 256
NCORES = 8
P = 128
RPC = N // NCORES   # 512 rows per core
RB = RPC // P       # 4 row-blocks per core
TJ = 2048           # max j-window width (buffer sizing)
WINDOWS = [(0, 1024), (1024, 1024), (2048, 2048)]  # (start, width)
NJC = len(WINDOWS)
NIT = RB * NJC      # main iterations (jc outer, rb inner)
NC5 = 512           # matmul moving-dim chunk
HD = 2              # d-halves (256 = 2*128)
NBSP = 2            # sp tile double-buffer depth
NBO = 3             # out tile buffer depth

F32 = mybir.dt.float32
BF16 = mybir.dt.bfloat16
ALU = mybir.AluOpType
ACTF = mybir.ActivationFunctionType

_compiled = {}


def _build_nc():
    nc = bass.Bass("TRN2")

    # xT: [256, 4096] bf16 viewed as [p, h, j] with d = h*128 + p
    xt_d = nc.declare_dram_parameter("xt", [D, N], BF16, isOutput=False)
    xbt_d = nc.declare_dram_parameter("xbt", [D, RPC], BF16, isOutput=False)
    w2c_d = nc.declare_dram_parameter("w2c", [D, 2], BF16, isOutput=False)
    bb_d = nc.declare_dram_parameter("bb", [1, 1], F32, isOutput=False)
    out_d = nc.declare_dram_parameter("out", [RPC, 4 * N], BF16, isOutput=True)
    out_d3 = out_d[:].rearrange("r (c n) -> r c n", c=4)
    xt_v = xt_d[:].rearrange("(h p) j -> p h j", p=P)     # [128, 2, 4096]
    xbt_v = xbt_d[:].rearrange("(h p) j -> p h j", p=P)   # [128, 2, 512]
    w2c_v = w2c_d[:].rearrange("(h p) c -> p h c", p=P)   # [128, 2, 2]

    from contextlib import ExitStack

    with ExitStack() as ctx:
        ec = ctx.enter_context
        # SBUF
        xt_sb = ec(nc.sbuf_tensor("xt_sb", [P, HD, N], BF16))
        xbt_sb = ec(nc.sbuf_tensor("xbt_sb", [P, HD, RPC], BF16))
        w2c_sb = ec(nc.sbuf_tensor("w2c_sb", [P, HD, 2], BF16))
        bb_col = ec(nc.sbuf_tensor("bb_col", [P, 1], F32))
        ones_l = ec(nc.sbuf_tensor("ones_l", [1, P], BF16))   # bcast lhsT
        oneone = ec(nc.sbuf_tensor("oneone", [1, 1], BF16))   # transpose rhs
        row_s = ec(nc.sbuf_tensor("row_s", [1, N], BF16))
        row_d = ec(nc.sbuf_tensor("row_d", [1, N], BF16))
        rown_s = ec(nc.sbuf_tensor("rown_s", [1, RPC], BF16))
        rown_d = ec(nc.sbuf_tensor("rown_d", [1, RPC], BF16))
        s_bf = ec(nc.sbuf_tensor("s_bf", [P, N], BF16))
        d_bf = ec(nc.sbuf_tensor("d_bf", [P, N], BF16))
        es = ec(nc.sbuf_tensor("es", [P, N], BF16))
        ed = ec(nc.sbuf_tensor("ed", [P, N], BF16))
        cols_bf = ec(nc.sbuf_tensor("cols_bf", [P, 2 * RB], F32))
        ecols = ec(nc.sbuf_tensor("ecols", [P, 2 * RB], F32))
        sp1 = [ec(nc.sbuf_tensor(f"sp1_{i}", [P, TJ], BF16)) for i in range(NBSP)]
        sp2 = [ec(nc.sbuf_tensor(f"sp2_{i}", [P, TJ], BF16)) for i in range(NBSP)]
        z1b = ec(nc.sbuf_tensor("z1b", [P, TJ], BF16))
        z2b = ec(nc.sbuf_tensor("z2b", [P, TJ], BF16))
        outb = [
            ec(nc.sbuf_tensor(f"outb{i}", [P, 4 * TJ], BF16)) for i in range(NBO)
        ]
        # PSUM: ps_a holds src-row then src-plane; ps_b dst-row/cols/plane
        ps_a = ec(nc.psum_tensor("ps_a", [P, TJ], F32))
        ps_b = ec(nc.psum_tensor("ps_b", [P, TJ], F32))
        # semaphores
        s_in = ec(nc.semaphore("s_in"))
        s_xt = ec(nc.semaphore("s_xt"))
        s_init = ec(nc.semaphore("s_init"))
        s_peown = ec(nc.semaphore("s_peown"))
        s_ro = ec(nc.semaphore("s_ro"))
        s_pebias = ec(nc.semaphore("s_pebias"))
        s_cols = ec(nc.semaphore("s_cols"))
        s_esc = ec(nc.semaphore("s_esc"))
        s_pj = ec(nc.semaphore("s_pj"))
        s_rows = ec(nc.semaphore("s_rows"))
        s_bc = ec(nc.semaphore("s_bc"))
        s_pl = ec(nc.semaphore("s_pl"))
        s_dpl = ec(nc.semaphore("s_dpl"))
        s_sp = ec(nc.semaphore("s_sp"))
        s_dve = ec(nc.semaphore("s_dve"))
        s_out = ec(nc.semaphore("s_out"))

        with nc.Block() as block:

            @block.gpsimd
            def _(g):
                g.memset(ones_l[:], 1.0)
                g.memset(oneone[:], 1.0).then_inc(s_init, 1)

            @block.sync
            def _(sy):
                sy.dma_start(out=w2c_sb[:], in_=w2c_v[:, :, :]).then_inc(s_in, 16)
                sy.dma_start(
                    out=bb_col[:],
                    in_=bb_d[0:1, :].partition_broadcast(P)[:, 0, :],
                ).then_inc(s_in, 16)
                sy.dma_start(out=xbt_sb[:], in_=xbt_v[:, :, :]).then_inc(s_in, 16)
                for j0, w in WINDOWS:
                    sy.dma_start(
                        out=xt_sb[:, :, j0 : j0 + w],
                        in_=xt_v[:, :, j0 : j0 + w],
                    ).then_inc(s_xt, 16)
                for it in range(NIT):
                    jc, rb = divmod(it, RB)
                    j0, w = WINDOWS[jc]
                    o = it % NBO
                    sy.wait_ge(s_dve, it + 1)
                    sy.dma_start(
                        out=out_d3[rb * P : (rb + 1) * P, :, j0 : j0 + w],
                        in_=outb[o][:, 0 : 4 * w].rearrange(
                            "p (c n) -> p c n", c=4
                        ),
                    ).then_inc(s_out, 16)
                sy.wait_ge(s_out, 16 * NIT)

            @block.tensor
            def _(t):
                t.wait_ge(s_in, 48)
                # own-row projections -> rows at partition 0 of ps_a / ps_b
                for h in range(HD):
                    nc.tensor.matmul(
                        ps_a[0:1, 0:RPC],
                        w2c_sb[:, h, 0:1],
                        xbt_sb[:, h, :],
                        start=(h == 0),
                        stop=(h == HD - 1),
                    )
                for h in range(HD):
                    ins = nc.tensor.matmul(
                        ps_b[0:1, 0:RPC],
                        w2c_sb[:, h, 1:2],
                        xbt_sb[:, h, :],
                        start=(h == 0),
                        stop=(h == HD - 1),
                    )
                ins.then_inc(s_peown, 1)
                # bias columns: [1,128] -> [128,1] transposes via ones matmul
                t.wait_ge(s_ro, 1)
                t.wait_ge(s_init, 1)
                for rb in range(RB):
                    nc.tensor.matmul(
                        ps_b[:, rb : rb + 1],
                        rown_s[0:1, rb * P : (rb + 1) * P],
                        oneone[:],
                    )
                for rb in range(RB):
                    ins = nc.tensor.matmul(
                        ps_b[:, RB + rb : RB + rb + 1],
                        rown_d[0:1, rb * P : (rb + 1) * P],
                        oneone[:],
                    )
                ins.then_inc(s_pebias, 1)
                for jc, (j0, w) in enumerate(WINDOWS):
                    nch = w // NC5
                    # projections for this window -> rows in ps_a/ps_b p0
                    t.wait_ge(s_xt, 16 * (jc + 1))
                    if jc == 0:
                        t.wait_ge(s_cols, 1)  # bias cols leave ps_b first
                    else:
                        t.wait_ge(s_pl, jc)   # planes of jc-1 consumed
                        t.wait_ge(s_dpl, jc)
                    for c in range(nch):
                        jsl = slice(j0 + c * NC5, j0 + (c + 1) * NC5)
                        csl = slice(c * NC5, (c + 1) * NC5)
                        for h in range(HD):
                            nc.tensor.matmul(
                                ps_a[0:1, csl],
                                w2c_sb[:, h, 0:1],
                                xt_sb[:, h, jsl],
                                start=(h == 0),
                                stop=(h == HD - 1),
                            )
                        for h in range(HD):
                            ins = nc.tensor.matmul(
                                ps_b[0:1, csl],
                                w2c_sb[:, h, 1:2],
                                xt_sb[:, h, jsl],
                                start=(h == 0),
                                stop=(h == HD - 1),
                            )
                    ins.then_inc(s_pj, 1)
                    # broadcast rows -> planes (overwrites ps_a/ps_b fully)
                    t.wait_ge(s_rows, jc + 1)
                    for c in range(nch):
                        jsl = slice(j0 + c * NC5, j0 + (c + 1) * NC5)
                        csl = slice(c * NC5, (c + 1) * NC5)
                        nc.tensor.matmul(
                            ps_a[:, csl], ones_l[:], row_s[0:1, jsl]
                        )
                        ins = nc.tensor.matmul(
                            ps_b[:, csl], ones_l[:], row_d[0:1, jsl]
                        )
                    ins.then_inc(s_bc, 1)

            @block.scalar
            def _(s):
                def rows(jc):
                    j0, w = WINDOWS[jc]
                    s.wait_ge(s_pj, jc + 1)
                    nc.scalar.copy(row_s[0:1, j0 : j0 + w], ps_a[0:1, 0:w])
                    nc.scalar.copy(
                        row_d[0:1, j0 : j0 + w], ps_b[0:1, 0:w]
                    ).then_inc(s_rows, 1)

                def planes(jc):
                    j0, w = WINDOWS[jc]
                    jsl = slice(j0, j0 + w)
                    s.wait_ge(s_bc, jc + 1)
                    nc.scalar.activation(
                        es[:, jsl], ps_a[:, 0:w], ACTF.Exp, bias=0.0, scale=1.0
                    )
                    nc.scalar.activation(
                        ed[:, jsl], ps_b[:, 0:w], ACTF.Exp, bias=0.0, scale=1.0
                    )
                    nc.scalar.copy(s_bf[:, jsl], ps_a[:, 0:w]).then_inc(s_pl, 1)

                # rows of the own projections -> SBUF (for PE transposes)
                s.wait_ge(s_peown, 1)
                nc.scalar.copy(rown_s[:], ps_a[0:1, 0:RPC])
                nc.scalar.copy(rown_d[:], ps_b[0:1, 0:RPC]).then_inc(s_ro, 1)
                # exp of bias cols
                s.wait_ge(s_cols, 1)
                nc.scalar.activation(
                    ecols[:], cols_bf[:], ACTF.Exp, bias=0.0, scale=1.0
                ).then_inc(s_esc, 1)
                for jc, (j0, w) in enumerate(WINDOWS):
                    jsl = slice(j0, j0 + w)
                    rows(jc)
                    planes(jc)
                    for rb in range(RB):
                        it = jc * RB + rb
                        b = it % NBSP
                        if it >= NBSP:
                            s.wait_ge(s_dve, it - NBSP + 1)
                        nc.scalar.activation(
                            sp1[b][:, 0:w], ed[:, jsl], ACTF.Ln,
                            bias=1.0, scale=ecols[:, rb : rb + 1],
                        )
                        nc.scalar.activation(
                            sp2[b][:, 0:w], es[:, jsl], ACTF.Ln,
                            bias=1.0, scale=ecols[:, RB + rb : RB + rb + 1],
                        ).then_inc(s_sp, 1)

            @block.vector
            def _(v):
                def dplane(jc):
                    j0, w = WINDOWS[jc]
                    v.wait_ge(s_bc, jc + 1)
                    nc.vector.tensor_copy(
                        d_bf[:, j0 : j0 + w], ps_b[:, 0:w]
                    ).then_inc(s_dpl, 1)

                # bias columns (+ b_src + b_dst)
                v.wait_ge(s_pebias, 1)
                nc.vector.tensor_scalar(
                    out=cols_bf[:], in0=ps_b[:, 0 : 2 * RB],
                    scalar1=bb_col[:, 0:1], scalar2=None, op0=ALU.add,
                ).then_inc(s_cols, 1)
                for jc, (j0, w) in enumerate(WINDOWS):
                    jsl = slice(j0, j0 + w)
                    dplane(jc)
                    v.wait_ge(s_pl, jc + 1)  # s_bf written by ACT
                    for rb in range(RB):
                        it = jc * RB + rb
                        b = it % NBSP
                        o = it % NBO
                        ot = outb[o]
                        p0 = ot[:, 0:w]
                        p1 = ot[:, w : 2 * w]
                        p2 = ot[:, 2 * w : 3 * w]
                        p3 = ot[:, 3 * w : 4 * w]
                        if it >= NBO:
                            v.wait_ge(s_out, 16 * (it - NBO + 1))
                        nc.vector.tensor_scalar(
                            out=z1b[:, 0:w], in0=d_bf[:, jsl],
                            scalar1=cols_bf[:, rb : rb + 1], scalar2=None,
                            op0=ALU.add,
                        )
                        nc.vector.tensor_scalar(
                            out=z2b[:, 0:w], in0=s_bf[:, jsl],
                            scalar1=cols_bf[:, RB + rb : RB + rb + 1],
                            scalar2=None, op0=ALU.add,
                        )
                        v.wait_ge(s_sp, it + 1)
                        nc.vector.tensor_tensor(
                            out=p0, in0=sp1[b][:, 0:w], in1=sp2[b][:, 0:w],
                            op=ALU.add,
                        )
                        nc.vector.tensor_tensor(
                            out=p1, in0=p0, in1=z1b[:, 0:w], op=ALU.subtract
                        )
                        nc.vector.tensor_tensor(
                            out=p3, in0=p0, in1=z2b[:, 0:w], op=ALU.subtract
                        )
                        nc.vector.tensor_tensor(
                            out=p2, in0=p1, in1=z2b[:, 0:w], op=ALU.subtract
                        ).then_inc(s_dve, 1)

    return nc


def _get_nc():
    if "nc" not in _compiled:
        _compiled["nc"] = _build_nc()
    return _compiled["nc"]


def _make_in_maps(inputs):
    x = np.asarray(inputs["x"], dtype=np.float32)
    w_src = np.asarray(inputs["w_src"], dtype=np.float32).reshape(1, D)
    w_dst = np.asarray(inputs["w_dst"], dtype=np.float32).reshape(1, D)
    b_src = np.asarray(inputs["b_src"], dtype=np.float32).reshape(-1)[0]
    b_dst = np.asarray(inputs["b_dst"], dtype=np.float32).reshape(-1)[0]
    xt = np.ascontiguousarray(x.T).astype(ml_dtypes.bfloat16)     # [256, 4096]
    w2c = np.ascontiguousarray(
        np.concatenate([w_src, w_dst], axis=0).T
    ).astype(ml_dtypes.bfloat16)                                  # [256, 2]
    bb = np.array([[np.float32(b_src) + np.float32(b_dst)]], dtype=np.float32)
    in_maps = []
    for m in range(NCORES):
        xbt = np.ascontiguousarray(xt[:, m * RPC : (m + 1) * RPC])
        in_maps.append({"xt": xt, "xbt": xbt, "w2c": w2c, "bb": bb})
    return in_maps


def _assemble(results):
    blocks = [
        np.asarray(results[m]["out"]).reshape(RPC, 4, N) for m in range(NCORES)
    ]
    full = np.concatenate(blocks, axis=0)                  # [N, 4, N] bf16
    full = full.transpose(0, 2, 1).astype(np.float32)      # [N, N, 4]
    return np.ascontiguousarray(-full).reshape(N * N, 4)


def kernel(**inputs) -> np.ndarray:
    nc = _get_nc()
    res = run_bass_kernel_spmd(nc, _make_in_maps(inputs), core_ids=list(range(NCORES)))
    return _assemble(res.results)


def kernel_traced(**inputs):
    """Like kernel() but also returns (output, exec_time_ns, profile_json)."""
    nc = _get_nc()
    res = run_bass_kernel_spmd(
        nc, _make_in_maps(inputs), core_ids=list(range(NCORES)), trace=True
    )
    return _assemble(res.results), res.exec_time_ns, res.profile_json


# revision 17
# speedup vs baseline: 1.0142x; 1.0142x over previous
"""Trainium2 Bass kernel for DecoderLinear_for_EffectiveLP_multiclass.

Math (reference):
    src = x @ w_src.T + b_src            # [N]
    dst = x @ w_dst.T + b_dst            # [N]
    s_ij = sigmoid(src[i] + dst[j])      # [N, N]
    channels: p_nb=(1-s_ij)(1-s_ji), p_pu=s_ij(1-s_ji),
              p_pb=s_ij*s_ji,        p_nu=(1-s_ij)s_ji
    out = log(clip(probs, 1e-10, 1))     # [N*N, 4]

On-device identities (clip never fires: |z| <= ~5 so min prob >> 1e-10):
    sp(z) = softplus(z);  z1 = src_i + dst_j;  z2 = dst_i + src_j
    -ch0 = sp1+sp2; -ch1 = sp1+sp2-z1; -ch3 = sp1+sp2-z2; -ch2 = sp1+sp2-z1-z2
The device writes the NEGATED channels in bf16; the host flips the sign
during the bf16->f32 conversion. The rel-err budget (2e-2) dwarfs bf16
rounding (~0.5% on the Frobenius norm).

Design (per core: 512 rows x 4096 j, 8 cores row-blockwise):
  - Host passes xT = x.T (bf16) so the projections run on the TENSOR engine:
    lhsT = w column [128d, 1], rhs = xT slice [128d, n<=512] -> PSUM row
    [1, n] accumulated over the two 128-d halves. No DVE reductions at all.
  - Row->plane broadcast is a single ones-vector matmul per 512-col chunk:
    out[p, j] = ones[1,p]^T @ row[1, j] (k=1). No transposes, no selectors.
  - softplus on ACT in ONE op per tile: sp1 = Ln(esc_rb * ed + 1) where
    ed = exp(dst-plane) (per-jc ACT Exp from PSUM) and esc_rb = exp(bias col)
    enters via the per-partition `scale` operand of ACTIVATE.
  - Per-core bias columns come from xbT (own 512 rows of xT): PE projects
    them to rows, then 8 tiny [1,128]->[128,1] transpose-matmuls make the
    per-partition columns. +(b_src+b_dst) is folded into the columns.
  - DVE per tile: 2x tensor_scalar (z1, z2; bf16 4x mode) + 4x tensor_tensor
    (bf16 2x mode) writing the four NEGATED channel planes contiguously.
  - Output tile [128, 4, 2048] bf16 (2 MiB) -> HBM; 16 tiles; jc-outer order
    so the j=0 planes gate only the first 4 tiles.

Measured op costs backing this schedule (this container, [128,2048] f32/bf16):
  DVE TT bf16 1211ns (2x), TS bf16 ~594ns (4x), STT always 1x (~2.3us);
  ACT 2.0-2.1us any func; PE matmul ~420ns + 320ns ldweights; per-core DMA
  ~335GB/s. Per-jc budget at 335GB/s is ~28us; DVE ~26.4, ACT ~26.3, PE ~18.
"""

import numpy as np
import ml_dtypes

import concourse.bass as bass
import concourse.mybir as mybir
from concourse.bass_utils import run_bass_kernel_spmd

N = 4096
D = 256
NCORES = 8
P = 128
RPC = N // NCORES   # 512 rows per core
RB = RPC // P       # 4 row-blocks per core
TJ = 2048           # max j-window width (buffer sizing)
WINDOWS = [(0, 2048), (2048, 2048)]  # (start, width)
NJC = len(WINDOWS)
NIT = RB * NJC      # main iterations (jc outer, rb inner)
NC5 = 512           # matmul moving-dim chunk
HD = 2              # d-halves (256 = 2*128)
NBSP = 3            # sp tile buffer depth
NBO = 3             # out tile buffer depth

F32 = mybir.dt.float32
BF16 = mybir.dt.bfloat16
ALU = mybir.AluOpType
ACTF = mybir.ActivationFunctionType

_compiled = {}


def _build_nc():
    nc = bass.Bass("TRN2")

    # xT: [256, 4096] bf16 viewed as [p, h, j] with d = h*128 + p
    xt_d = nc.declare_dram_parameter("xt", [D, N], BF16, isOutput=False)
    xbt_d = nc.declare_dram_parameter("xbt", [D, RPC], BF16, isOutput=False)
    w2c_d = nc.declare_dram_parameter("w2c", [D, 2], BF16, isOutput=False)
    bb_d = nc.declare_dram_parameter("bb", [1, 1], F32, isOutput=False)
    out_d = nc.declare_dram_parameter("out", [RPC, 4 * N], BF16, isOutput=True)
    out_d3 = out_d[:].rearrange("r (c n) -> r c n", c=4)
    xt_v = xt_d[:].rearrange("(h p) j -> p h j", p=P)     # [128, 2, 4096]
    xbt_v = xbt_d[:].rearrange("(h p) j -> p h j", p=P)   # [128, 2, 512]
    w2c_v = w2c_d[:].rearrange("(h p) c -> p h c", p=P)   # [128, 2, 2]

    from contextlib import ExitStack

    with ExitStack() as ctx:
        ec = ctx.enter_context
        # SBUF
        xt_sb = ec(nc.sbuf_tensor("xt_sb", [P, HD, N], BF16))
        xbt_sb = ec(nc.sbuf_tensor("xbt_sb", [P, HD, RPC], BF16))
        w2c_sb = ec(nc.sbuf_tensor("w2c_sb", [P, HD, 2], BF16))
        bb_col = ec(nc.sbuf_tensor("bb_col", [P, 1], F32))
        ones_l = ec(nc.sbuf_tensor("ones_l", [1, P], BF16))   # bcast lhsT
        oneone = ec(nc.sbuf_tensor("oneone", [1, 1], BF16))   # transpose rhs
        row_s = ec(nc.sbuf_tensor("row_s", [1, N], BF16))
        row_d = ec(nc.sbuf_tensor("row_d", [1, N], BF16))
        rown_s = ec(nc.sbuf_tensor("rown_s", [1, RPC], BF16))
        rown_d = ec(nc.sbuf_tensor("rown_d", [1, RPC], BF16))
        s_bf = ec(nc.sbuf_tensor("s_bf", [P, N], BF16))
        d_bf = ec(nc.sbuf_tensor("d_bf", [P, N], BF16))
        es = ec(nc.sbuf_tensor("es", [P, N], BF16))
        ed = ec(nc.sbuf_tensor("ed", [P, N], BF16))
        cols_bf = ec(nc.sbuf_tensor("cols_bf", [P, 2 * RB], F32))
        ecols = ec(nc.sbuf_tensor("ecols", [P, 2 * RB], F32))
        sp1 = [ec(nc.sbuf_tensor(f"sp1_{i}", [P, TJ], BF16)) for i in range(NBSP)]
        sp2 = [ec(nc.sbuf_tensor(f"sp2_{i}", [P, TJ], BF16)) for i in range(NBSP)]
        z1b = ec(nc.sbuf_tensor("z1b", [P, TJ], BF16))
        z2b = ec(nc.sbuf_tensor("z2b", [P, TJ], BF16))
        outb = [
            ec(nc.sbuf_tensor(f"outb{i}", [P, 4 * TJ], BF16)) for i in range(NBO)
        ]
        # PSUM: ps_a holds src-row then src-plane; ps_b dst-row/cols/plane
        ps_a = ec(nc.psum_tensor("ps_a", [P, TJ], F32))
        ps_b = ec(nc.psum_tensor("ps_b", [P, TJ], F32))
        # semaphores
        s_in = ec(nc.semaphore("s_in"))
        s_xt = ec(nc.semaphore("s_xt"))
        s_init = ec(nc.semaphore("s_init"))
        s_peown = ec(nc.semaphore("s_peown"))
        s_ro = ec(nc.semaphore("s_ro"))
        s_pebias = ec(nc.semaphore("s_pebias"))
        s_cols = ec(nc.semaphore("s_cols"))
        s_esc = ec(nc.semaphore("s_esc"))
        s_pj = ec(nc.semaphore("s_pj"))
        s_rows = ec(nc.semaphore("s_rows"))
        s_bc = ec(nc.semaphore("s_bc"))
        s_pl = ec(nc.semaphore("s_pl"))
        s_dpl = ec(nc.semaphore("s_dpl"))
        s_sp = ec(nc.semaphore("s_sp"))
        s_dve = ec(nc.semaphore("s_dve"))
        s_out = ec(nc.semaphore("s_out"))

        with nc.Block() as block:

            @block.gpsimd
            def _(g):
                g.memset(ones_l[:], 1.0)
                g.memset(oneone[:], 1.0).then_inc(s_init, 1)

            @block.sync
            def _(sy):
                sy.dma_start(out=w2c_sb[:], in_=w2c_v[:, :, :]).then_inc(s_in, 16)
                sy.dma_start(
                    out=bb_col[:],
                    in_=bb_d[0:1, :].partition_broadcast(P)[:, 0, :],
                ).then_inc(s_in, 16)
                sy.dma_start(out=xbt_sb[:], in_=xbt_v[:, :, :]).then_inc(s_in, 16)
                sy.dma_start(
                    out=xt_sb[:, :, 0:1024], in_=xt_v[:, :, 0:1024]
                ).then_inc(s_xt, 16)
                sy.dma_start(
                    out=xt_sb[:, :, 1024:2048], in_=xt_v[:, :, 1024:2048]
                ).then_inc(s_xt, 16)
                for j0, w in WINDOWS[1:]:
                    sy.dma_start(
                        out=xt_sb[:, :, j0 : j0 + w],
                        in_=xt_v[:, :, j0 : j0 + w],
                    ).then_inc(s_xt, 16)
                for it in range(NIT):
                    jc, rb = divmod(it, RB)
                    j0, w = WINDOWS[jc]
                    o = it % NBO
                    sy.wait_ge(s_dve, it + 1)
                    sy.dma_start(
                        out=out_d3[rb * P : (rb + 1) * P, :, j0 : j0 + w],
                        in_=outb[o][:, 0 : 4 * w].rearrange(
                            "p (c n) -> p c n", c=4
                        ),
                    ).then_inc(s_out, 16)
                sy.wait_ge(s_out, 16 * NIT)

            @block.tensor
            def _(t):
                t.wait_ge(s_in, 48)
                # own-row projections -> rows at partition 0 of ps_a / ps_b
                for h in range(HD):
                    nc.tensor.matmul(
                        ps_a[0:1, 0:RPC],
                        w2c_sb[:, h, 0:1],
                        xbt_sb[:, h, :],
                        start=(h == 0),
                        stop=(h == HD - 1),
                    )
                for h in range(HD):
                    ins = nc.tensor.matmul(
                        ps_b[0:1, 0:RPC],
                        w2c_sb[:, h, 1:2],
                        xbt_sb[:, h, :],
                        start=(h == 0),
                        stop=(h == HD - 1),
                    )
                ins.then_inc(s_peown, 1)
                for jc, (j0, w) in enumerate(WINDOWS):
                    nch = w // NC5
                    # projections for this window -> rows in ps_a/ps_b p0
                    if jc == 0:
                        t.wait_ge(s_xt, 16)
                        t.wait_ge(s_ro, 1)    # own rows leave ps_a/ps_b
                    else:
                        t.wait_ge(s_xt, 16 * (jc + 2))
                        t.wait_ge(s_pl, jc)   # planes of jc-1 consumed
                        t.wait_ge(s_dpl, jc)
                    for c in range(nch):
                        if jc == 0 and c == 2:
                            t.wait_ge(s_xt, 32)
                        jsl = slice(j0 + c * NC5, j0 + (c + 1) * NC5)
                        csl = slice(c * NC5, (c + 1) * NC5)
                        for h in range(HD):
                            nc.tensor.matmul(
                                ps_a[0:1, csl],
                                w2c_sb[:, h, 0:1],
                                xt_sb[:, h, jsl],
                                start=(h == 0),
                                stop=(h == HD - 1),
                            )
                        for h in range(HD):
                            ins = nc.tensor.matmul(
                                ps_b[0:1, csl],
                                w2c_sb[:, h, 1:2],
                                xt_sb[:, h, jsl],
                                start=(h == 0),
                                stop=(h == HD - 1),
                            )
                    ins.then_inc(s_pj, 1)
                    if jc == 0:
                        # bias cols: [1,128] -> [128,1] transposes; needs
                        # rows(0) copied out (s_rows) since they share ps_b
                        t.wait_ge(s_init, 1)
                        t.wait_ge(s_rows, 1)
                        for rb in range(RB):
                            nc.tensor.matmul(
                                ps_b[:, rb : rb + 1],
                                rown_s[0:1, rb * P : (rb + 1) * P],
                                oneone[:],
                            )
                        for rb in range(RB):
                            ins = nc.tensor.matmul(
                                ps_b[:, RB + rb : RB + rb + 1],
                                rown_d[0:1, rb * P : (rb + 1) * P],
                                oneone[:],
                            )
                        ins.then_inc(s_pebias, 1)
                        t.wait_ge(s_cols, 1)  # cols leave ps_b before bcast
                    # broadcast rows -> planes (overwrites ps_a/ps_b fully)
                    t.wait_ge(s_rows, jc + 1)
                    for c in range(nch):
                        jsl = slice(j0 + c * NC5, j0 + (c + 1) * NC5)
                        csl = slice(c * NC5, (c + 1) * NC5)
                        nc.tensor.matmul(
                            ps_a[:, csl], ones_l[:], row_s[0:1, jsl]
                        )
                        ins = nc.tensor.matmul(
                            ps_b[:, csl], ones_l[:], row_d[0:1, jsl]
                        )
                    ins.then_inc(s_bc, 1)

            @block.scalar
            def _(s):
                def rows(jc):
                    j0, w = WINDOWS[jc]
                    s.wait_ge(s_pj, jc + 1)
                    nc.scalar.copy(row_s[0:1, j0 : j0 + w], ps_a[0:1, 0:w])
                    nc.scalar.copy(
                        row_d[0:1, j0 : j0 + w], ps_b[0:1, 0:w]
                    ).then_inc(s_rows, 1)

                def planes(jc):
                    j0, w = WINDOWS[jc]
                    jsl = slice(j0, j0 + w)
                    s.wait_ge(s_bc, jc + 1)
                    nc.scalar.activation(
                        es[:, jsl], ps_a[:, 0:w], ACTF.Exp, bias=0.0, scale=1.0
                    )
                    nc.scalar.activation(
                        ed[:, jsl], ps_b[:, 0:w], ACTF.Exp, bias=0.0, scale=1.0
                    )
                    nc.scalar.copy(s_bf[:, jsl], ps_a[:, 0:w]).then_inc(s_pl, 1)

                # rows of the own projections -> SBUF (for PE transposes)
                s.wait_ge(s_peown, 1)
                nc.scalar.copy(rown_s[:], ps_a[0:1, 0:RPC])
                nc.scalar.copy(rown_d[:], ps_b[0:1, 0:RPC]).then_inc(s_ro, 1)
                for jc, (j0, w) in enumerate(WINDOWS):
                    jsl = slice(j0, j0 + w)
                    rows(jc)
                    planes(jc)
                    if jc == 0:
                        # exp of bias cols (after rows(0): s_cols depends on it)
                        s.wait_ge(s_cols, 1)
                        nc.scalar.activation(
                            ecols[:], cols_bf[:], ACTF.Exp, bias=0.0, scale=1.0
                        ).then_inc(s_esc, 1)
                    for rb in range(RB):
                        it = jc * RB + rb
                        b = it % NBSP
                        if it >= NBSP:
                            s.wait_ge(s_dve, it - NBSP + 1)
                        nc.scalar.activation(
                            sp1[b][:, 0:w], ed[:, jsl], ACTF.Ln,
                            bias=1.0, scale=ecols[:, rb : rb + 1],
                        )
                        nc.scalar.activation(
                            sp2[b][:, 0:w], es[:, jsl], ACTF.Ln,
                            bias=1.0, scale=ecols[:, RB + rb : RB + rb + 1],
                        ).then_inc(s_sp, 1)

            @block.vector
            def _(v):
                def dplane(jc):
                    j0, w = WINDOWS[jc]
                    v.wait_ge(s_bc, jc + 1)
                    nc.vector.tensor_copy(
                        d_bf[:, j0 : j0 + w], ps_b[:, 0:w]
                    ).then_inc(s_dpl, 1)

                # bias columns (+ b_src + b_dst)
                v.wait_ge(s_pebias, 1)
                nc.vector.tensor_scalar(
                    out=cols_bf[:], in0=ps_b[:, 0 : 2 * RB],
                    scalar1=bb_col[:, 0:1], scalar2=None, op0=ALU.add,
                ).then_inc(s_cols, 1)
                for jc, (j0, w) in enumerate(WINDOWS):
                    jsl = slice(j0, j0 + w)
                    dplane(jc)
                    v.wait_ge(s_pl, jc + 1)  # s_bf written by ACT
                    for rb in range(RB):
                        it = jc * RB + rb
                        b = it % NBSP
                        o = it % NBO
                        ot = outb[o]
                        p0 = ot[:, 0:w]
                        p1 = ot[:, w : 2 * w]
                        p2 = ot[:, 2 * w : 3 * w]
                        p3 = ot[:, 3 * w : 4 * w]
                        if it >= NBO:
                            v.wait_ge(s_out, 16 * (it - NBO + 1))
                        nc.vector.tensor_scalar(
                            out=z1b[:, 0:w], in0=d_bf[:, jsl],
                            scalar1=cols_bf[:, rb : rb + 1], scalar2=None,
                            op0=ALU.add,
                        )
                        nc.vector.tensor_scalar(
                            out=z2b[:, 0:w], in0=s_bf[:, jsl],
                            scalar1=cols_bf[:, RB + rb : RB + rb + 1],
                            scalar2=None, op0=ALU.add,
                        )
                        v.wait_ge(s_sp, it + 1)
                        nc.vector.tensor_tensor(
                            out=p0, in0=sp1[b][:, 0:w], in1=sp2[b][:, 0:w],
                            op=ALU.add,
                        )
                        nc.vector.tensor_tensor(
                            out=p1, in0=p0, in1=z1b[:, 0:w], op=ALU.subtract
                        )
                        nc.vector.tensor_tensor(
                            out=p3, in0=p0, in1=z2b[:, 0:w], op=ALU.subtract
                        )
                        nc.vector.tensor_tensor(
                            out=p2, in0=p1, in1=z2b[:, 0:w], op=ALU.subtract
                        ).then_inc(s_dve, 1)

    return nc


def _get_nc():
    if "nc" not in _compiled:
        _compiled["nc"] = _build_nc()
    return _compiled["nc"]


def _make_in_maps(inputs):
    x = np.asarray(inputs["x"], dtype=np.float32)
    w_src = np.asarray(inputs["w_src"], dtype=np.float32).reshape(1, D)
    w_dst = np.asarray(inputs["w_dst"], dtype=np.float32).reshape(1, D)
    b_src = np.asarray(inputs["b_src"], dtype=np.float32).reshape(-1)[0]
    b_dst = np.asarray(inputs["b_dst"], dtype=np.float32).reshape(-1)[0]
    xt = np.ascontiguousarray(x.T).astype(ml_dtypes.bfloat16)     # [256, 4096]
    w2c = np.ascontiguousarray(
        np.concatenate([w_src, w_dst], axis=0).T
    ).astype(ml_dtypes.bfloat16)                                  # [256, 2]
    bb = np.array([[np.float32(b_src) + np.float32(b_dst)]], dtype=np.float32)
    in_maps = []
    for m in range(NCORES):
        xbt = np.ascontiguousarray(xt[:, m * RPC : (m + 1) * RPC])
        in_maps.append({"xt": xt, "xbt": xbt, "w2c": w2c, "bb": bb})
    return in_maps


def _assemble(results):
    blocks = [
        np.asarray(results[m]["out"]).reshape(RPC, 4, N) for m in range(NCORES)
    ]
    full = np.concatenate(blocks, axis=0)                  # [N, 4, N] bf16
    full = full.transpose(0, 2, 1).astype(np.float32)      # [N, N, 4]
    return np.ascontiguousarray(-full).reshape(N * N, 4)


def kernel(**inputs) -> np.ndarray:
    nc = _get_nc()
    res = run_bass_kernel_spmd(nc, _make_in_maps(inputs), core_ids=list(range(NCORES)))
    return _assemble(res.results)


def kernel_traced(**inputs):
    """Like kernel() but also returns (output, exec_time_ns, profile_json)."""
    nc = _get_nc()
    res = run_bass_kernel_spmd(
        nc, _make_in_maps(inputs), core_ids=list(range(NCORES)), trace=True
    )
    return _assemble(res.results), res.exec_time_ns, res.profile_json


# revision 20
# speedup vs baseline: 1.1474x; 1.1313x over previous
"""Trainium2 Bass kernel for DecoderLinear_for_EffectiveLP_multiclass.

Math (reference):
    src = x @ w_src.T + b_src            # [N]
    dst = x @ w_dst.T + b_dst            # [N]
    s_ij = sigmoid(src[i] + dst[j])      # [N, N]
    channels: p_nb=(1-s_ij)(1-s_ji), p_pu=s_ij(1-s_ji),
              p_pb=s_ij*s_ji,        p_nu=(1-s_ij)s_ji
    out = log(clip(probs, 1e-10, 1))     # [N*N, 4]

On-device identities (clip never fires: |z| <= ~5 so min prob >> 1e-10):
    sp(z) = softplus(z);  z1 = src_i + dst_j;  z2 = dst_i + src_j
    -ch0 = sp1+sp2; -ch1 = sp1+sp2-z1; -ch3 = sp1+sp2-z2; -ch2 = sp1+sp2-z1-z2
The device writes the NEGATED channels in bf16; the host flips the sign
during the bf16->f32 conversion. The rel-err budget (2e-2) dwarfs bf16
rounding (~0.5% on the Frobenius norm).

Design (per core: 512 rows x 4096 j, 8 cores row-blockwise):
  - Host passes xT = x.T (bf16) so the projections run on the TENSOR engine:
    lhsT = w column [128d, 1], rhs = xT slice [128d, n<=512] -> PSUM row
    [1, n] accumulated over the two 128-d halves. No DVE reductions at all.
  - Row->plane broadcast is a single ones-vector matmul per 512-col chunk:
    out[p, j] = ones[1,p]^T @ row[1, j] (k=1). No transposes, no selectors.
  - softplus on ACT in ONE op per tile: sp1 = Ln(esc_rb * ed + 1) where
    ed = exp(dst-plane) (per-jc ACT Exp from PSUM) and esc_rb = exp(bias col)
    enters via the per-partition `scale` operand of ACTIVATE.
  - Per-core bias columns come from xbT (own 512 rows of xT): PE projects
    them to rows, then 8 tiny [1,128]->[128,1] transpose-matmuls make the
    per-partition columns. +(b_src+b_dst) is folded into the columns.
  - DVE per tile: 2x tensor_scalar (z1, z2; bf16 4x mode) + 4x tensor_tensor
    (bf16 2x mode) writing the four NEGATED channel planes contiguously.
  - Output tile [128, 4, 2048] bf16 (2 MiB) -> HBM; 16 tiles; jc-outer order
    so the j=0 planes gate only the first 4 tiles.

Measured op costs backing this schedule (this container, [128,2048] f32/bf16):
  DVE TT bf16 1211ns (2x), TS bf16 ~594ns (4x), STT always 1x (~2.3us);
  ACT 2.0-2.1us any func; PE matmul ~420ns + 320ns ldweights; per-core DMA
  ~335GB/s. Per-jc budget at 335GB/s is ~28us; DVE ~26.4, ACT ~26.3, PE ~18.
"""

import numpy as np
import ml_dtypes

import concourse.bass as bass
import concourse.mybir as mybir
from concourse.bass_utils import run_bass_kernel_spmd

N = 4096
D = 256
NCORES = 8
P = 128
RPC = N // NCORES   # 512 rows per core
RB = RPC // P       # 4 row-blocks per core
TJ = 2048           # max j-window width (buffer sizing)
WINDOWS = [(0, 2048), (2048, 2048)]  # (start, width)
NJC = len(WINDOWS)
NIT = RB * NJC      # main iterations (jc outer, rb inner)
NC5 = 512           # matmul moving-dim chunk
HD = 2              # d-halves (256 = 2*128)
NBSP = 2            # sp tile double-buffer depth
NBO = 3             # out tile buffer depth

F32 = mybir.dt.float32
BF16 = mybir.dt.bfloat16
ALU = mybir.AluOpType
ACTF = mybir.ActivationFunctionType

_compiled = {}


def _build_nc():
    nc = bass.Bass("TRN2")

    # xT: [256, 4096] bf16 viewed as [p, h, j] with d = h*128 + p
    xt_d = nc.declare_dram_parameter("xt", [D, N], BF16, isOutput=False)
    xbt_d = nc.declare_dram_parameter("xbt", [D, RPC], BF16, isOutput=False)
    w2c_d = nc.declare_dram_parameter("w2c", [D, 2], BF16, isOutput=False)
    wrep_d = nc.declare_dram_parameter("wrep", [2 * D, P], BF16, isOutput=False)
    bb_d = nc.declare_dram_parameter("bb", [1, 1], F32, isOutput=False)
    out_d = nc.declare_dram_parameter("out", [RPC, 4 * N], BF16, isOutput=True)
    out_d3 = out_d[:].rearrange("r (c n) -> r c n", c=4)
    xt_v = xt_d[:].rearrange("(h p) j -> p h j", p=P)     # [128, 2, 4096]
    xbt_v = xbt_d[:].rearrange("(h p) j -> p h j", p=P)   # [128, 2, 512]
    w2c_v = w2c_d[:].rearrange("(h p) c -> p h c", p=P)   # [128, 2, 2]
    wrep_v = wrep_d[:].rearrange("(c h p) m -> p c h m", c=2, h=HD)

    from contextlib import ExitStack

    with ExitStack() as ctx:
        ec = ctx.enter_context
        # SBUF
        xt_sb = ec(nc.sbuf_tensor("xt_sb", [P, HD, N], BF16))
        xbt_sb = ec(nc.sbuf_tensor("xbt_sb", [P, HD, RPC], BF16))
        w2c_sb = ec(nc.sbuf_tensor("w2c_sb", [P, HD, 2], BF16))
        bb_col = ec(nc.sbuf_tensor("bb_col", [P, 1], F32))
        oneone = ec(nc.sbuf_tensor("oneone", [1, 1], BF16))   # transpose rhs
        wrep_sb = ec(nc.sbuf_tensor("wrep_sb", [P, 2, HD, P], BF16))
        rown_s = ec(nc.sbuf_tensor("rown_s", [1, RPC], BF16))
        rown_d = ec(nc.sbuf_tensor("rown_d", [1, RPC], BF16))
        s_bf = ec(nc.sbuf_tensor("s_bf", [P, N], BF16))
        d_bf = ec(nc.sbuf_tensor("d_bf", [P, N], BF16))
        es = ec(nc.sbuf_tensor("es", [P, N], BF16))
        ed = ec(nc.sbuf_tensor("ed", [P, N], BF16))
        cols_bf = ec(nc.sbuf_tensor("cols_bf", [P, 2 * RB], F32))
        ecols = ec(nc.sbuf_tensor("ecols", [P, 2 * RB], F32))
        sp1 = [ec(nc.sbuf_tensor(f"sp1_{i}", [P, TJ], BF16)) for i in range(NBSP)]
        sp2 = [ec(nc.sbuf_tensor(f"sp2_{i}", [P, TJ], BF16)) for i in range(NBSP)]
        z1b = ec(nc.sbuf_tensor("z1b", [P, TJ], BF16))
        z2b = ec(nc.sbuf_tensor("z2b", [P, TJ], BF16))
        outb = [
            ec(nc.sbuf_tensor(f"outb{i}", [P, 4 * TJ], BF16)) for i in range(NBO)
        ]
        # PSUM: ps_a holds src-row then src-plane; ps_b dst-row/cols/plane
        ps_a = ec(nc.psum_tensor("ps_a", [P, TJ], F32))
        ps_b = ec(nc.psum_tensor("ps_b", [P, TJ], F32))
        # semaphores
        s_in = ec(nc.semaphore("s_in"))
        s_xt = ec(nc.semaphore("s_xt"))
        s_init = ec(nc.semaphore("s_init"))
        s_peown = ec(nc.semaphore("s_peown"))
        s_ro = ec(nc.semaphore("s_ro"))
        s_pebias = ec(nc.semaphore("s_pebias"))
        s_cols = ec(nc.semaphore("s_cols"))
        s_esc = ec(nc.semaphore("s_esc"))
        s_pj = ec(nc.semaphore("s_pj"))
        s_rows = ec(nc.semaphore("s_rows"))
        s_bc = ec(nc.semaphore("s_bc"))
        s_pl = ec(nc.semaphore("s_pl"))
        s_dpl = ec(nc.semaphore("s_dpl"))
        s_sp = ec(nc.semaphore("s_sp"))
        s_dve = ec(nc.semaphore("s_dve"))
        s_out = ec(nc.semaphore("s_out"))

        with nc.Block() as block:

            @block.gpsimd
            def _(g):
                g.memset(oneone[:], 1.0).then_inc(s_init, 1)

            @block.sync
            def _(sy):
                sy.dma_start(out=w2c_sb[:], in_=w2c_v[:, :, :]).then_inc(s_in, 16)
                sy.dma_start(
                    out=bb_col[:],
                    in_=bb_d[0:1, :].partition_broadcast(P)[:, 0, :],
                ).then_inc(s_in, 16)
                sy.dma_start(out=xbt_sb[:], in_=xbt_v[:, :, :]).then_inc(s_in, 16)
                sy.dma_start(out=wrep_sb[:], in_=wrep_v[:, :, :, :]).then_inc(
                    s_in, 16
                )
                for j0, w in WINDOWS:
                    sy.dma_start(
                        out=xt_sb[:, :, j0 : j0 + w],
                        in_=xt_v[:, :, j0 : j0 + w],
                    ).then_inc(s_xt, 16)
                for it in range(NIT):
                    jc, rb = divmod(it, RB)
                    j0, w = WINDOWS[jc]
                    o = it % NBO
                    sy.wait_ge(s_dve, it + 1)
                    sy.dma_start(
                        out=out_d3[rb * P : (rb + 1) * P, :, j0 : j0 + w],
                        in_=outb[o][:, 0 : 4 * w].rearrange(
                            "p (c n) -> p c n", c=4
                        ),
                    ).then_inc(s_out, 16)
                sy.wait_ge(s_out, 16 * NIT)

            @block.tensor
            def _(t):
                t.wait_ge(s_in, 64)
                # own-row projections -> rows at partition 0 of ps_a / ps_b
                for h in range(HD):
                    nc.tensor.matmul(
                        ps_a[0:1, 0:RPC],
                        w2c_sb[:, h, 0:1],
                        xbt_sb[:, h, :],
                        start=(h == 0),
                        stop=(h == HD - 1),
                    )
                for h in range(HD):
                    ins = nc.tensor.matmul(
                        ps_b[0:1, 0:RPC],
                        w2c_sb[:, h, 1:2],
                        xbt_sb[:, h, :],
                        start=(h == 0),
                        stop=(h == HD - 1),
                    )
                ins.then_inc(s_peown, 1)
                # bias columns: [1,128] -> [128,1] transposes via ones matmul
                t.wait_ge(s_ro, 1)
                t.wait_ge(s_init, 1)
                for rb in range(RB):
                    nc.tensor.matmul(
                        ps_b[:, rb : rb + 1],
                        rown_s[0:1, rb * P : (rb + 1) * P],
                        oneone[:],
                    )
                for rb in range(RB):
                    ins = nc.tensor.matmul(
                        ps_b[:, RB + rb : RB + rb + 1],
                        rown_d[0:1, rb * P : (rb + 1) * P],
                        oneone[:],
                    )
                ins.then_inc(s_pebias, 1)
                for jc, (j0, w) in enumerate(WINDOWS):
                    nch = w // NC5
                    # broadcast planes DIRECTLY: out[p,j] = sum_d w[d]*xT[d,j]
                    # via replicated-weight stationary [128d, 128p]
                    t.wait_ge(s_xt, 16 * (jc + 1))
                    if jc == 0:
                        t.wait_ge(s_cols, 1)  # bias cols leave ps_b first
                    else:
                        t.wait_ge(s_pl, jc)   # planes of jc-1 consumed
                        t.wait_ge(s_dpl, jc)
                    for c in range(nch):
                        jsl = slice(j0 + c * NC5, j0 + (c + 1) * NC5)
                        csl = slice(c * NC5, (c + 1) * NC5)
                        for h in range(HD):
                            nc.tensor.matmul(
                                ps_a[:, csl],
                                wrep_sb[:, 0, h, :],
                                xt_sb[:, h, jsl],
                                start=(h == 0),
                                stop=(h == HD - 1),
                            )
                        for h in range(HD):
                            ins = nc.tensor.matmul(
                                ps_b[:, csl],
                                wrep_sb[:, 1, h, :],
                                xt_sb[:, h, jsl],
                                start=(h == 0),
                                stop=(h == HD - 1),
                            )
                    ins.then_inc(s_bc, 1)

            @block.scalar
            def _(s):
                def planes(jc):
                    j0, w = WINDOWS[jc]
                    jsl = slice(j0, j0 + w)
                    s.wait_ge(s_bc, jc + 1)
                    nc.scalar.activation(
                        es[:, jsl], ps_a[:, 0:w], ACTF.Exp, bias=0.0, scale=1.0
                    )
                    nc.scalar.activation(
                        ed[:, jsl], ps_b[:, 0:w], ACTF.Exp, bias=0.0, scale=1.0
                    )
                    nc.scalar.copy(s_bf[:, jsl], ps_a[:, 0:w]).then_inc(s_pl, 1)

                # rows of the own projections -> SBUF (for PE transposes)
                s.wait_ge(s_peown, 1)
                nc.scalar.copy(rown_s[:], ps_a[0:1, 0:RPC])
                nc.scalar.copy(rown_d[:], ps_b[0:1, 0:RPC]).then_inc(s_ro, 1)
                # exp of bias cols
                s.wait_ge(s_cols, 1)
                nc.scalar.activation(
                    ecols[:], cols_bf[:], ACTF.Exp, bias=0.0, scale=1.0
                ).then_inc(s_esc, 1)
                for jc, (j0, w) in enumerate(WINDOWS):
                    jsl = slice(j0, j0 + w)
                    planes(jc)
                    for rb in range(RB):
                        it = jc * RB + rb
                        b = it % NBSP
                        if it >= NBSP:
                            s.wait_ge(s_dve, it - NBSP + 1)
                        nc.scalar.activation(
                            sp1[b][:, 0:w], ed[:, jsl], ACTF.Ln,
                            bias=1.0, scale=ecols[:, rb : rb + 1],
                        )
                        nc.scalar.activation(
                            sp2[b][:, 0:w], es[:, jsl], ACTF.Ln,
                            bias=1.0, scale=ecols[:, RB + rb : RB + rb + 1],
                        ).then_inc(s_sp, 1)

            @block.vector
            def _(v):
                def dplane(jc):
                    j0, w = WINDOWS[jc]
                    v.wait_ge(s_bc, jc + 1)
                    nc.vector.tensor_copy(
                        d_bf[:, j0 : j0 + w], ps_b[:, 0:w]
                    ).then_inc(s_dpl, 1)

                # bias columns (+ b_src + b_dst)
                v.wait_ge(s_pebias, 1)
                nc.vector.tensor_scalar(
                    out=cols_bf[:], in0=ps_b[:, 0 : 2 * RB],
                    scalar1=bb_col[:, 0:1], scalar2=None, op0=ALU.add,
                ).then_inc(s_cols, 1)
                for jc, (j0, w) in enumerate(WINDOWS):
                    jsl = slice(j0, j0 + w)
                    dplane(jc)
                    v.wait_ge(s_pl, jc + 1)  # s_bf written by ACT
                    for rb in range(RB):
                        it = jc * RB + rb
                        b = it % NBSP
                        o = it % NBO
                        ot = outb[o]
                        p0 = ot[:, 0:w]
                        p1 = ot[:, w : 2 * w]
                        p2 = ot[:, 2 * w : 3 * w]
                        p3 = ot[:, 3 * w : 4 * w]
                        if it >= NBO:
                            v.wait_ge(s_out, 16 * (it - NBO + 1))
                        nc.vector.tensor_scalar(
                            out=z1b[:, 0:w], in0=d_bf[:, jsl],
                            scalar1=cols_bf[:, rb : rb + 1], scalar2=None,
                            op0=ALU.add,
                        )
                        nc.vector.tensor_scalar(
                            out=z2b[:, 0:w], in0=s_bf[:, jsl],
                            scalar1=cols_bf[:, RB + rb : RB + rb + 1],
                            scalar2=None, op0=ALU.add,
                        )
                        v.wait_ge(s_sp, it + 1)
                        nc.vector.tensor_tensor(
                            out=p0, in0=sp1[b][:, 0:w], in1=sp2[b][:, 0:w],
                            op=ALU.add,
                        )
                        nc.vector.tensor_tensor(
                            out=p1, in0=p0, in1=z1b[:, 0:w], op=ALU.subtract
                        )
                        nc.vector.tensor_tensor(
                            out=p3, in0=p0, in1=z2b[:, 0:w], op=ALU.subtract
                        )
                        nc.vector.tensor_tensor(
                            out=p2, in0=p1, in1=z2b[:, 0:w], op=ALU.subtract
                        ).then_inc(s_dve, 1)

    return nc


def _get_nc():
    if "nc" not in _compiled:
        _compiled["nc"] = _build_nc()
    return _compiled["nc"]


def _make_in_maps(inputs):
    x = np.asarray(inputs["x"], dtype=np.float32)
    w_src = np.asarray(inputs["w_src"], dtype=np.float32).reshape(1, D)
    w_dst = np.asarray(inputs["w_dst"], dtype=np.float32).reshape(1, D)
    b_src = np.asarray(inputs["b_src"], dtype=np.float32).reshape(-1)[0]
    b_dst = np.asarray(inputs["b_dst"], dtype=np.float32).reshape(-1)[0]
    xt = np.ascontiguousarray(x.T).astype(ml_dtypes.bfloat16)     # [256, 4096]
    w2c = np.ascontiguousarray(
        np.concatenate([w_src, w_dst], axis=0).T
    ).astype(ml_dtypes.bfloat16)                                  # [256, 2]
    bb = np.array([[np.float32(b_src) + np.float32(b_dst)]], dtype=np.float32)
    # wrep[c, h, p, m] = w[c][h*128+p] for all m (stationary replicated cols)
    w2rows = np.concatenate([w_src, w_dst], axis=0)            # [2, 256] f32
    wrep = np.ascontiguousarray(
        np.broadcast_to(
            w2rows.reshape(2, HD, P, 1), (2, HD, P, P)
        ).reshape(2 * D, P)
    ).astype(ml_dtypes.bfloat16)
    in_maps = []
    for m in range(NCORES):
        xbt = np.ascontiguousarray(xt[:, m * RPC : (m + 1) * RPC])
        in_maps.append(
            {"xt": xt, "xbt": xbt, "w2c": w2c, "wrep": wrep, "bb": bb}
        )
    return in_maps


def _assemble(results):
    blocks = [
        np.asarray(results[m]["out"]).reshape(RPC, 4, N) for m in range(NCORES)
    ]
    full = np.concatenate(blocks, axis=0)                  # [N, 4, N] bf16
    full = full.transpose(0, 2, 1).astype(np.float32)      # [N, N, 4]
    return np.ascontiguousarray(-full).reshape(N * N, 4)


def kernel(**inputs) -> np.ndarray:
    nc = _get_nc()
    res = run_bass_kernel_spmd(nc, _make_in_maps(inputs), core_ids=list(range(NCORES)))
    return _assemble(res.results)


def kernel_traced(**inputs):
    """Like kernel() but also returns (output, exec_time_ns, profile_json)."""
    nc = _get_nc()
    res = run_bass_kernel_spmd(
        nc, _make_in_maps(inputs), core_ids=list(range(NCORES)), trace=True
    )
    return _assemble(res.results), res.exec_time_ns, res.profile_json
